# revision 4
# baseline (speedup 1.0000x reference)
"""Causal self-attention (B=2, S=2048, D=1024, H=16) on 8 TRN2 NeuronCores.

Sharding: core c handles batch b = c//4 and head group g = c%4 (4 heads).
Per-core kernel computes, for its 4 heads:
  Q^T, K^T : [d_local, S] layouts (d on partitions)  via  W^T-slice.T @ x^T
  V_aug    : [S, 4*65]  natural layout + ones column per head (for rowsums)
  scores^T : [k, q] tiles = K_h^T.T @ Q_h^T   (contraction over head_dim=64)
  exp      : ACT Exp eviction with scale=1/8 folded in; causal masking via
             0/1 mask multiply on the 4 diagonal tiles; upper tiles skipped
             entirely (output buffers are zero-initialized by the runtime)
  ctx^T,rowsum : accumulated [65, q] PSUM via V_aug.T @ exp
  normalize: recip(rowsum) broadcast across partitions with a K=1 matmul,
             then DVE multiplies (attn in [k,q] layout -> host transposes)
  out^T    : partial output projection Wo-slice @ ctx (host sums the 4
             head-group partials per batch)

Outputs per core: attnT [4, S, S] ([k,q] layout) and outT [D, S].
"""
import sys

for _p in ("/opt/trn_rl_repo",):
    if _p not in sys.path:
        sys.path.append(_p)

from contextlib import ExitStack

import numpy as np

import concourse.bacc as bacc
import concourse.mybir as mybir
import concourse.tile as tile
from concourse.bass_utils import run_bass_kernel_spmd

F32 = mybir.dt.float32
AF = mybir.ActivationFunctionType

B, S, D, H, HD = 2, 2048, 1024, 16, 64
HL = 4            # heads per core
DL = HL * HD      # 256 local head dims per core
P = 128
QB = 512          # q block width
NQB = S // QB     # 4
NKT = S // P      # 16 k tiles
NEB = D // P      # 8 e blocks (contraction tiles for projections)
SCALE = 1.0 / float(np.sqrt(HD))

_CACHE = {}


def _build():
    if "nc" in _CACHE:
        return _CACHE["nc"]
    nc = bacc.Bacc(None, target_bir_lowering=False)

    xT_d = nc.dram_tensor("xT", [D, S], F32, kind="ExternalInput")
    wqT_d = nc.dram_tensor("wqT", [D, DL], F32, kind="ExternalInput")
    wkT_d = nc.dram_tensor("wkT", [D, DL], F32, kind="ExternalInput")
    wvT_d = nc.dram_tensor("wvT", [D, DL], F32, kind="ExternalInput")
    woT_d = nc.dram_tensor("woT", [DL, D], F32, kind="ExternalInput")
    bq_d = nc.dram_tensor("bq", [DL, 1], F32, kind="ExternalInput")
    bk_d = nc.dram_tensor("bk", [DL, 1], F32, kind="ExternalInput")
    bv_d = nc.dram_tensor("bv", [P, DL], F32, kind="ExternalInput")
    bo_d = nc.dram_tensor("bo", [D, 1], F32, kind="ExternalInput")
    masks_d = nc.dram_tensor("masks", [4, P, QB], F32, kind="ExternalInput")
    attnT_d = nc.dram_tensor("attnT", [HL, S, S], F32, kind="ExternalOutput")
    outT_d = nc.dram_tensor("outT", [D, S], F32, kind="ExternalOutput")

    with tile.TileContext(nc) as tc, ExitStack() as ctx:
        const = ctx.enter_context(tc.tile_pool(name="const", bufs=1))
        xpool = ctx.enter_context(tc.tile_pool(name="xp", bufs=12))
        expp = ctx.enter_context(tc.tile_pool(name="expp", bufs=2))
        outp = ctx.enter_context(tc.tile_pool(name="outp", bufs=2))
        psA = ctx.enter_context(tc.tile_pool(name="psA", bufs=2, space="PSUM"))
        psB = ctx.enter_context(tc.tile_pool(name="psB", bufs=3, space="PSUM"))

        wq_sb = const.tile([P, NEB, DL], F32, tag="wq")
        wk_sb = const.tile([P, NEB, DL], F32, tag="wk")
        wv_sb = const.tile([P, NEB, DL], F32, tag="wv")
        wo_sb = const.tile([P, 2, D], F32, tag="wo")
        nc.sync.dma_start(out=wq_sb[:], in_=wqT_d[:].rearrange("(a p) d -> p a d", p=P))
        nc.sync.dma_start(out=wk_sb[:], in_=wkT_d[:].rearrange("(a p) d -> p a d", p=P))
        nc.sync.dma_start(out=wv_sb[:], in_=wvT_d[:].rearrange("(a p) d -> p a d", p=P))
        nc.sync.dma_start(out=wo_sb[:], in_=woT_d[:].rearrange("(a p) e -> p a e", p=P))

        bq_sb = const.tile([P, 2, 1], F32, tag="bq")
        bk_sb = const.tile([P, 2, 1], F32, tag="bk")
        bv_sb = const.tile([P, DL], F32, tag="bv")
        bo_sb = const.tile([P, NEB, 1], F32, tag="bo")
        nc.sync.dma_start(out=bq_sb[:], in_=bq_d[:].rearrange("(a p) o -> p a o", p=P))
        nc.sync.dma_start(out=bk_sb[:], in_=bk_d[:].rearrange("(a p) o -> p a o", p=P))
        nc.sync.dma_start(out=bv_sb[:], in_=bv_d[:])
        nc.sync.dma_start(out=bo_sb[:], in_=bo_d[:].rearrange("(a p) o -> p a o", p=P))

        mask_sb = const.tile([P, 4, QB], F32, tag="mask")
        nc.sync.dma_start(out=mask_sb[:], in_=masks_d[:].rearrange("j p q -> p j q"))
        ones_sb = const.tile([1, P], F32, tag="ones")
        nc.vector.memset(ones_sb[:], 1.0)

        # persistent activations: heads {2t, 2t+1} live in partition halves of tile t
        qt_sb = [const.tile([P, S], F32, tag=f"qt{t}", name=f"qt{t}") for t in range(2)]
        kt_sb = [const.tile([P, S], F32, tag=f"kt{t}", name=f"kt{t}") for t in range(2)]
        v_sb = const.tile([P, NKT, HL * 65], F32, tag="vaug")
        ctx_sb = [const.tile([P, S], F32, tag=f"ctx{t}", name=f"ctx{t}") for t in range(2)]

        # ---- Phase 1: projections ----
        for sb in range(NQB):
            xts = []
            for eb in range(NEB):
                xt = xpool.tile([P, QB], F32, tag="xt")
                nc.sync.dma_start(
                    out=xt[:], in_=xT_d[eb * P : (eb + 1) * P, sb * QB : (sb + 1) * QB]
                )
                xts.append(xt)
            for w_sb_, b_sb_, dest in ((wq_sb, bq_sb, qt_sb), (wk_sb, bk_sb, kt_sb)):
                ps = psA.tile([P, 2, QB], F32, tag="A")
                for dt in range(2):
                    for eb in range(NEB):
                        nc.tensor.matmul(
                            ps[:, dt, :],
                            w_sb_[:, eb, dt * P : (dt + 1) * P],
                            xts[eb][:],
                            start=(eb == 0),
                            stop=(eb == NEB - 1),
                        )
                for dt in range(2):
                    nc.scalar.activation(
                        dest[dt][:, sb * QB : (sb + 1) * QB],
                        ps[:, dt, :],
                        AF.Identity,
                        bias=b_sb_[:, dt, :],
                        scale=1.0,
                    )
            psv = psA.tile([P, 4, DL], F32, tag="A")
            for c in range(4):
                for eb in range(NEB):
                    nc.tensor.matmul(
                        psv[:, c, :],
                        xts[eb][:, c * P : (c + 1) * P],
                        wv_sb[:, eb, :],
                        start=(eb == 0),
                        stop=(eb == NEB - 1),
                    )
            for c in range(4):
                st = sb * 4 + c
                vdst = v_sb[:, st, :].rearrange("p (h x) -> p h x", x=65)
                nc.vector.tensor_add(
                    vdst[:, :, 0:64],
                    psv[:, c, :].rearrange("p (h x) -> p h x", x=64),
                    bv_sb[:].rearrange("p (h x) -> p h x", x=64),
                )
                nc.vector.memset(vdst[:, :, 64:65], 1.0)

        # ---- Phase 2: attention ----
        for h in range(HL):
            th, oh = h // 2, (h % 2) * 64
            for qi in range(NQB):
                nt = 4 * (qi + 1)
                exp_b = expp.tile([P, NKT, QB], F32, tag="exp")
                pctx = psB.tile([P, QB], F32, tag="B")
                rhs_q = qt_sb[th][oh : oh + 64, qi * QB : (qi + 1) * QB]
                for kg in range((nt + 1) // 2):
                    k0 = 2 * kg
                    k1 = min(2 * kg + 1, nt - 1)
                    n = k1 - k0 + 1
                    pss = psA.tile([P, 2, QB], F32, tag="A")
                    for kt in range(k0, k1 + 1):
                        nc.tensor.matmul(
                            pss[:, kt - k0, :],
                            kt_sb[th][oh : oh + 64, kt * P : (kt + 1) * P],
                            rhs_q,
                            start=True,
                            stop=True,
                        )
                    nc.scalar.activation(
                        exp_b[:, k0 : k0 + n, :], pss[:, 0:n, :], AF.Exp, scale=SCALE
                    )
                    for kt in range(k0, k1 + 1):
                        j = kt - 4 * qi
                        if 0 <= j <= 3:
                            nc.vector.tensor_mul(
                                exp_b[:, kt, :], exp_b[:, kt, :], mask_sb[:, j, :]
                            )
                    for kt in range(k0, k1 + 1):
                        nc.tensor.matmul(
                            pctx[0:65, :],
                            v_sb[:, kt, h * 65 : (h + 1) * 65],
                            exp_b[:, kt, :],
                            start=(kt == 0),
                            stop=(kt == nt - 1),
                        )
                rrow = outp.tile([1, QB], F32, tag="rr")
                nc.vector.reciprocal(rrow[0:1, :], pctx[64:65, :])
                prb = psB.tile([P, QB], F32, tag="B")
                nc.tensor.matmul(prb[:], ones_sb[:], rrow[0:1, :], start=True, stop=True)
                rb_sb = outp.tile([P, QB], F32, tag="rb")
                nc.vector.tensor_copy(rb_sb[:], prb[:])
                nc.vector.tensor_mul(
                    ctx_sb[th][oh : oh + 64, qi * QB : (qi + 1) * QB],
                    pctx[0:64, :],
                    rb_sb[0:64, :],
                )
                nc.vector.tensor_mul(
                    exp_b[:, 0:nt, :],
                    exp_b[:, 0:nt, :],
                    rb_sb[:, None, :].broadcast_to([P, nt, QB]),
                )
                for kt in range(nt):
                    j = kt - 4 * qi
                    qs = 128 * j if j > 0 else 0
                    nc.sync.dma_start(
                        out=attnT_d[
                            h, kt * P : (kt + 1) * P, qi * QB + qs : (qi + 1) * QB
                        ],
                        in_=exp_b[:, kt, qs:QB],
                    )

        # ---- Phase 3: output projection (partial over local head dims) ----
        for et in range(NEB):
            for q2 in range(NQB):
                po = psA.tile([P, 2, QB], F32, tag="A")
                for dt in range(2):
                    nc.tensor.matmul(
                        po[:, 0, :],
                        wo_sb[:, dt, et * P : (et + 1) * P],
                        ctx_sb[dt][:, q2 * QB : (q2 + 1) * QB],
                        start=(dt == 0),
                        stop=(dt == 1),
                    )
                st = outp.tile([P, QB], F32, tag="ot")
                nc.scalar.activation(
                    st[:], po[:, 0, :], AF.Identity, bias=bo_sb[:, et, :], scale=1.0
                )
                nc.sync.dma_start(
                    out=outT_d[et * P : (et + 1) * P, q2 * QB : (q2 + 1) * QB],
                    in_=st[:],
                )

    nc.compile()
    _CACHE["nc"] = nc
    return nc


def _masks_host():
    k = np.arange(P).reshape(P, 1)
    q = np.arange(QB).reshape(1, QB)
    return np.stack(
        [(q >= k + P * j).astype(np.float32) for j in range(4)], axis=0
    )


def _shard_inputs(x, Wq, bq, Wk, bk, Wv, bv, Wo, bo):
    masks = _masks_host()
    in_maps = []
    for c in range(8):
        b, g = divmod(c, 4)
        sl = slice(g * DL, (g + 1) * DL)
        in_maps.append(
            {
                "xT": np.ascontiguousarray(x[b].T).astype(np.float32),
                "wqT": np.ascontiguousarray(Wq[sl].T).astype(np.float32),
                "wkT": np.ascontiguousarray(Wk[sl].T).astype(np.float32),
                "wvT": np.ascontiguousarray(Wv[sl].T).astype(np.float32),
                "woT": np.ascontiguousarray(Wo[:, sl].T).astype(np.float32),
                "bq": np.ascontiguousarray(bq[sl].reshape(DL, 1)).astype(np.float32),
                "bk": np.ascontiguousarray(bk[sl].reshape(DL, 1)).astype(np.float32),
                "bv": np.ascontiguousarray(
                    np.broadcast_to(bv[sl], (P, DL))
                ).astype(np.float32),
                "bo": np.ascontiguousarray(
                    (bo if g == 0 else np.zeros_like(bo)).reshape(D, 1)
                ).astype(np.float32),
                "masks": masks,
            }
        )
    return in_maps


def run(inputs, trace=False):
    nc = _build()
    in_maps = _shard_inputs(**inputs)
    res = run_bass_kernel_spmd(nc, in_maps, list(range(8)), trace=trace)
    attn = np.empty((B, H, S, S), np.float32)
    out = np.zeros((B, S, D), np.float32)
    for c in range(8):
        b, g = divmod(c, 4)
        r = res.results[c]
        attn[b, g * HL : (g + 1) * HL] = r["attnT"].transpose(0, 2, 1)
        out[b] += r["outT"].T
    return (out, attn), res


def kernel(**inputs):
    outputs, _ = run(inputs, trace=False)
    return outputs


# revision 5
# speedup vs baseline: 1.7744x; 1.7744x over previous
"""Causal self-attention (B=2, S=2048, D=1024, H=16) on 8 TRN2 NeuronCores.

Sharding: core c handles batch b = c//4 and head group g = c%4 (4 heads).
Per-core kernel computes, for its 4 heads:
  Q^T, K^T : [d_local, S] layouts (d on partitions)  via  W^T-slice.T @ x^T
  V_aug    : [S, 4*65]  natural layout + ones column per head (for rowsums)
  scores^T : [k, q] tiles = K_h^T.T @ Q_h^T   (contraction over head_dim=64)
  exp      : ACT Exp eviction (scale=1/8 folded) -> bf16; causal masking via
             0/1 mask multiply on the 4 diagonal tiles; upper tiles skipped
             entirely (output buffers are zero-initialized by the runtime)
  ctx^T,rowsum : accumulated [65, q] PSUM via V_aug.T @ exp (fp32 accum)
  normalize: recip(rowsum) broadcast across partitions with a K=1 matmul,
             then DVE multiplies producing fp32 attn tiles ([k,q] layout ->
             host transposes) and bf16 ctx
  out^T    : partial output projection Wo-slice @ ctx (host sums the 4
             head-group partials per batch)

Matmul operands are bf16 (fp32 PSUM accumulation); softmax arithmetic and
all outputs are fp32.

Outputs per core: attnT [4, S, S] ([k,q] layout) and outT [D, S].
"""
import sys

for _p in ("/opt/trn_rl_repo",):
    if _p not in sys.path:
        sys.path.append(_p)

from contextlib import ExitStack

import ml_dtypes
import numpy as np

import concourse.bacc as bacc
import concourse.mybir as mybir
import concourse.tile as tile
from concourse.bass_utils import run_bass_kernel_spmd

F32 = mybir.dt.float32
BF16 = mybir.dt.bfloat16
AF = mybir.ActivationFunctionType

B, S, D, H, HD = 2, 2048, 1024, 16, 64
HL = 4            # heads per core
DL = HL * HD      # 256 local head dims per core
P = 128
QB = 512          # q block width
NQB = S // QB     # 4
NKT = S // P      # 16 k tiles
NEB = D // P      # 8 e blocks (contraction tiles for projections)
SCALE = 1.0 / float(np.sqrt(HD))

_CACHE = {}


def _build():
    if "nc" in _CACHE:
        return _CACHE["nc"]
    nc = bacc.Bacc(None, target_bir_lowering=False)

    xT_d = nc.dram_tensor("xT", [D, S], BF16, kind="ExternalInput")
    wqT_d = nc.dram_tensor("wqT", [D, DL], BF16, kind="ExternalInput")
    wkT_d = nc.dram_tensor("wkT", [D, DL], BF16, kind="ExternalInput")
    wvT_d = nc.dram_tensor("wvT", [D, DL], BF16, kind="ExternalInput")
    woT_d = nc.dram_tensor("woT", [DL, D], BF16, kind="ExternalInput")
    bq_d = nc.dram_tensor("bq", [DL, 1], F32, kind="ExternalInput")
    bk_d = nc.dram_tensor("bk", [DL, 1], F32, kind="ExternalInput")
    bv_d = nc.dram_tensor("bv", [P, DL], F32, kind="ExternalInput")
    bo_d = nc.dram_tensor("bo", [D, 1], F32, kind="ExternalInput")
    masks_d = nc.dram_tensor("masks", [4, P, QB], BF16, kind="ExternalInput")
    attnT_d = nc.dram_tensor("attnT", [HL, S, S], F32, kind="ExternalOutput")
    outT_d = nc.dram_tensor("outT", [D, S], F32, kind="ExternalOutput")

    with tile.TileContext(nc) as tc, ExitStack() as ctx:
        const = ctx.enter_context(tc.tile_pool(name="const", bufs=1))
        xpool = ctx.enter_context(tc.tile_pool(name="xp", bufs=12))
        expp = ctx.enter_context(tc.tile_pool(name="expp", bufs=2))
        astg = ctx.enter_context(tc.tile_pool(name="astg", bufs=6))
        outp = ctx.enter_context(tc.tile_pool(name="outp", bufs=2))
        psA = ctx.enter_context(tc.tile_pool(name="psA", bufs=2, space="PSUM"))
        psB = ctx.enter_context(tc.tile_pool(name="psB", bufs=3, space="PSUM"))

        wq_sb = const.tile([P, NEB, DL], BF16, tag="wq")
        wk_sb = const.tile([P, NEB, DL], BF16, tag="wk")
        wv_sb = const.tile([P, NEB, DL], BF16, tag="wv")
        wo_sb = const.tile([P, 2, D], BF16, tag="wo")
        nc.sync.dma_start(out=wq_sb[:], in_=wqT_d[:].rearrange("(a p) d -> p a d", p=P))
        nc.sync.dma_start(out=wk_sb[:], in_=wkT_d[:].rearrange("(a p) d -> p a d", p=P))
        nc.sync.dma_start(out=wv_sb[:], in_=wvT_d[:].rearrange("(a p) d -> p a d", p=P))
        nc.sync.dma_start(out=wo_sb[:], in_=woT_d[:].rearrange("(a p) e -> p a e", p=P))

        bq_sb = const.tile([P, 2, 1], F32, tag="bq")
        bk_sb = const.tile([P, 2, 1], F32, tag="bk")
        bv_sb = const.tile([P, DL], F32, tag="bv")
        bo_sb = const.tile([P, NEB, 1], F32, tag="bo")
        nc.sync.dma_start(out=bq_sb[:], in_=bq_d[:].rearrange("(a p) o -> p a o", p=P))
        nc.sync.dma_start(out=bk_sb[:], in_=bk_d[:].rearrange("(a p) o -> p a o", p=P))
        nc.sync.dma_start(out=bv_sb[:], in_=bv_d[:])
        nc.sync.dma_start(out=bo_sb[:], in_=bo_d[:].rearrange("(a p) o -> p a o", p=P))

        mask_sb = const.tile([P, 4, QB], BF16, tag="mask")
        nc.sync.dma_start(out=mask_sb[:], in_=masks_d[:].rearrange("j p q -> p j q"))
        ones_sb = const.tile([1, P], F32, tag="ones")
        nc.vector.memset(ones_sb[:], 1.0)

        # persistent activations: heads {2t, 2t+1} live in partition halves of tile t
        qt_sb = [const.tile([P, S], BF16, tag=f"qt{t}", name=f"qt{t}") for t in range(2)]
        kt_sb = [const.tile([P, S], BF16, tag=f"kt{t}", name=f"kt{t}") for t in range(2)]
        v_sb = const.tile([P, NKT, HL * 65], BF16, tag="vaug")
        ctx_sb = [const.tile([P, S], BF16, tag=f"ctx{t}", name=f"ctx{t}") for t in range(2)]

        # ---- Phase 1: projections ----
        for sb in range(NQB):
            xts = []
            for eb in range(NEB):
                xt = xpool.tile([P, QB], BF16, tag="xt")
                nc.sync.dma_start(
                    out=xt[:], in_=xT_d[eb * P : (eb + 1) * P, sb * QB : (sb + 1) * QB]
                )
                xts.append(xt)
            for w_sb_, b_sb_, dest in ((wq_sb, bq_sb, qt_sb), (wk_sb, bk_sb, kt_sb)):
                ps = psA.tile([P, 2, QB], F32, tag="A")
                for dt in range(2):
                    for eb in range(NEB):
                        nc.tensor.matmul(
                            ps[:, dt, :],
                            w_sb_[:, eb, dt * P : (dt + 1) * P],
                            xts[eb][:],
                            start=(eb == 0),
                            stop=(eb == NEB - 1),
                        )
                for dt in range(2):
                    nc.scalar.activation(
                        dest[dt][:, sb * QB : (sb + 1) * QB],
                        ps[:, dt, :],
                        AF.Identity,
                        bias=b_sb_[:, dt, :],
                        scale=1.0,
                    )
            psv = psA.tile([P, 4, DL], F32, tag="A")
            for c in range(4):
                for eb in range(NEB):
                    nc.tensor.matmul(
                        psv[:, c, :],
                        xts[eb][:, c * P : (c + 1) * P],
                        wv_sb[:, eb, :],
                        start=(eb == 0),
                        stop=(eb == NEB - 1),
                    )
            for c in range(4):
                st = sb * 4 + c
                vdst = v_sb[:, st, :].rearrange("p (h x) -> p h x", x=65)
                nc.vector.tensor_add(
                    vdst[:, :, 0:64],
                    psv[:, c, :].rearrange("p (h x) -> p h x", x=64),
                    bv_sb[:].rearrange("p (h x) -> p h x", x=64),
                )
                nc.vector.memset(vdst[:, :, 64:65], 1.0)

        # ---- Phase 2: attention ----
        for h in range(HL):
            th, oh = h // 2, (h % 2) * 64
            for qi in range(NQB):
                nt = 4 * (qi + 1)
                exp_b = expp.tile([P, NKT, QB], BF16, tag="exp")
                pctx = psB.tile([P, QB], F32, tag="B")
                rhs_q = qt_sb[th][oh : oh + 64, qi * QB : (qi + 1) * QB]
                for kg in range((nt + 1) // 2):
                    k0 = 2 * kg
                    k1 = min(2 * kg + 1, nt - 1)
                    n = k1 - k0 + 1
                    pss = psA.tile([P, 2, QB], F32, tag="A")
                    for kt in range(k0, k1 + 1):
                        nc.tensor.matmul(
                            pss[:, kt - k0, :],
                            kt_sb[th][oh : oh + 64, kt * P : (kt + 1) * P],
                            rhs_q,
                            start=True,
                            stop=True,
                        )
                    nc.scalar.activation(
                        exp_b[:, k0 : k0 + n, :], pss[:, 0:n, :], AF.Exp, scale=SCALE
                    )
                    for kt in range(k0, k1 + 1):
                        j = kt - 4 * qi
                        if 0 <= j <= 3:
                            nc.vector.tensor_mul(
                                exp_b[:, kt, :], exp_b[:, kt, :], mask_sb[:, j, :]
                            )
                    for kt in range(k0, k1 + 1):
                        nc.tensor.matmul(
                            pctx[0:65, :],
                            v_sb[:, kt, h * 65 : (h + 1) * 65],
                            exp_b[:, kt, :],
                            start=(kt == 0),
                            stop=(kt == nt - 1),
                        )
                rrow = outp.tile([1, QB], F32, tag="rr")
                nc.vector.reciprocal(rrow[0:1, :], pctx[64:65, :])
                prb = psB.tile([P, QB], F32, tag="B")
                nc.tensor.matmul(prb[:], ones_sb[:], rrow[0:1, :], start=True, stop=True)
                rb_sb = outp.tile([P, QB], F32, tag="rb")
                nc.vector.tensor_copy(rb_sb[:], prb[:])
                nc.vector.tensor_mul(
                    ctx_sb[th][oh : oh + 64, qi * QB : (qi + 1) * QB],
                    pctx[0:64, :],
                    rb_sb[0:64, :],
                )
                for kg in range((nt + 1) // 2):
                    k0 = 2 * kg
                    k1 = min(2 * kg + 1, nt - 1)
                    n = k1 - k0 + 1
                    at = astg.tile([P, 2, QB], F32, tag="astg")
                    nc.vector.tensor_mul(
                        at[:, 0:n, :],
                        exp_b[:, k0 : k0 + n, :],
                        rb_sb[:, None, :].broadcast_to([P, n, QB]),
                    )
                    for kt in range(k0, k1 + 1):
                        j = kt - 4 * qi
                        qs = 128 * j if j > 0 else 0
                        nc.sync.dma_start(
                            out=attnT_d[
                                h, kt * P : (kt + 1) * P, qi * QB + qs : (qi + 1) * QB
                            ],
                            in_=at[:, kt - k0, qs:QB],
                        )

        # ---- Phase 3: output projection (partial over local head dims) ----
        for et in range(NEB):
            for q2 in range(NQB):
                po = psA.tile([P, 2, QB], F32, tag="A")
                for dt in range(2):
                    nc.tensor.matmul(
                        po[:, 0, :],
                        wo_sb[:, dt, et * P : (et + 1) * P],
                        ctx_sb[dt][:, q2 * QB : (q2 + 1) * QB],
                        start=(dt == 0),
                        stop=(dt == 1),
                    )
                st = outp.tile([P, QB], F32, tag="ot")
                nc.scalar.activation(
                    st[:], po[:, 0, :], AF.Identity, bias=bo_sb[:, et, :], scale=1.0
                )
                nc.sync.dma_start(
                    out=outT_d[et * P : (et + 1) * P, q2 * QB : (q2 + 1) * QB],
                    in_=st[:],
                )

    nc.compile()
    _CACHE["nc"] = nc
    return nc


def _masks_host():
    k = np.arange(P).reshape(P, 1)
    q = np.arange(QB).reshape(1, QB)
    return np.stack(
        [(q >= k + P * j) for j in range(4)], axis=0
    ).astype(ml_dtypes.bfloat16)


def _bf(a):
    return np.ascontiguousarray(a).astype(ml_dtypes.bfloat16)


def _shard_inputs(x, Wq, bq, Wk, bk, Wv, bv, Wo, bo):
    masks = _masks_host()
    in_maps = []
    for c in range(8):
        b, g = divmod(c, 4)
        sl = slice(g * DL, (g + 1) * DL)
        in_maps.append(
            {
                "xT": _bf(x[b].T),
                "wqT": _bf(Wq[sl].T),
                "wkT": _bf(Wk[sl].T),
                "wvT": _bf(Wv[sl].T),
                "woT": _bf(Wo[:, sl].T),
                "bq": np.ascontiguousarray(bq[sl].reshape(DL, 1)).astype(np.float32),
                "bk": np.ascontiguousarray(bk[sl].reshape(DL, 1)).astype(np.float32),
                "bv": np.ascontiguousarray(
                    np.broadcast_to(bv[sl], (P, DL))
                ).astype(np.float32),
                "bo": np.ascontiguousarray(
                    (bo if g == 0 else np.zeros_like(bo)).reshape(D, 1)
                ).astype(np.float32),
                "masks": masks,
            }
        )
    return in_maps


def run(inputs, trace=False):
    nc = _build()
    in_maps = _shard_inputs(**inputs)
    res = run_bass_kernel_spmd(nc, in_maps, list(range(8)), trace=trace)
    attn = np.empty((B, H, S, S), np.float32)
    out = np.zeros((B, S, D), np.float32)
    for c in range(8):
        b, g = divmod(c, 4)
        r = res.results[c]
        attn[b, g * HL : (g + 1) * HL] = r["attnT"].transpose(0, 2, 1)
        out[b] += r["outT"].T
    return (out, attn), res


def kernel(**inputs):
    outputs, _ = run(inputs, trace=False)
    return outputs
